# revision 1
# baseline (speedup 1.0000x reference)
"""Trainium2 Bass kernel for nn_CTCConsistencyLoss_7310034338203.

Strategy (data-parallel over batch, 8 cores x 4 samples):
  logits = pred @ W.T + b  ->  E = exp(logits)  ->  p = E * exp(C1 - logZ)
  CTC forward DP in the *linear* domain with three tricks that keep
  everything in f32 range and one instruction-pair per time step:
    - lattice tilt: shift-by-k transition weights carry e^{k*CTILT};
      corrected exactly at readout (path tilt telescopes to c*(s_end-s_start))
    - per-frame boost e^{C1} (Z-normalized probs), corrected via C1*in_len
    - periodic exact rescale by 1/sum(alpha) every 64 steps (log tracked)
  Blocked state layout: 9 chunks of 48 states (+16 halo) per sample;
  alpha-block (rows 0:64) and beta-block (rows 64:128, skip-masked) stacked
  along K so each DP step is ONE 128x128 matmul + ONE vector multiply
  against pre-gathered per-state probability tiles (bf16).
  Halo refreshed every 8 steps by a permutation matmul; snapshots of the
  readout chunks for t>=799 support per-sample input_lengths; final
  state/time selection via host-built one-hot matmuls on device.

Host side builds only index-derived 0/1 one-hots and constants from the
int inputs; all floating-point compute runs on the NeuronCores.
"""
import numpy as np
import ml_dtypes

B, T, D, V, L = 32, 1000, 768, 31, 200
S = 2 * L + 1
NJ = 9
CORE = 48
HALO = 16
BLK = 64
B_LOC = 4
NCOL = B_LOC * NJ            # 36
C1 = 2.5
CTILT = -0.9
RESC = 64                    # rescale period
REFRESH = 8                  # halo refresh period
SNAP_T0 = 799
NT = T - SNAP_T0             # 201 snapshot slots
HC = NT * B_LOC * 3          # 2412 history columns
N_RESC = (T - 1) // RESC     # 15
NPB = np.float32


def _build_core_consts(tgt2d, in_len, tgt_len, b_base):
    """Host-built constants for one core (samples b_base..b_base+3)."""
    il = in_len[b_base:b_base + B_LOC].astype(np.int64)
    tl = tgt_len[b_base:b_base + B_LOC].astype(np.int64)
    ext = np.zeros((B_LOC, S), np.int64)
    ext[:, 1::2] = tgt2d[b_base:b_base + B_LOC]
    skip = np.zeros((B_LOC, S), bool)
    skip[:, 2:] = (ext[:, 2:] != 0) & (ext[:, 2:] != ext[:, :-2])
    m2s = np.zeros((B_LOC, S + 2), bool)
    m2s[:, :S - 2] = skip[:, 2:]

    # gather one-hots, bf16 lhsT tiles (31, 128) packed g-major (g = b*9+j)
    # rows 0:64 -> p, rows 64:128 -> p*skip_shifted; states > 2*tl truncated
    G = np.zeros((V, NCOL, 128), np.float32)
    for b in range(B_LOC):
        for j in range(NJ):
            for m in range(BLK):
                s = CORE * j - HALO + m
                if 0 <= s <= 2 * int(tl[b]):
                    G[ext[b, s], b * NJ + j, m] = 1.0
                    if m2s[b, s]:
                        G[ext[b, s], b * NJ + j, 64 + m] = 1.0
    g_in = G.reshape(V, NCOL * 128).astype(ml_dtypes.bfloat16)

    e1, e2 = np.exp(CTILT), np.exp(2 * CTILT)
    W1 = np.zeros((128, 128), np.float32)
    for m in range(BLK):
        W1[m, m] = 1.0
        if m >= 1:
            W1[m - 1, m] = e1
        if m >= 2:
            W1[64 + m - 2, m] = e2
        W1[:, 64 + m] = W1[:, m]

    PR = np.zeros((128, 80), np.float32)
    for m in range(16):
        PR[48 + m, m] = 1.0
        PR[112 + m, 64 + m] = 1.0

    OH01 = np.zeros((128, NCOL), np.float32)
    for b in range(B_LOC):
        c = b * NJ
        OH01[16, c] = 1.0
        OH01[17, c] = e1
        OH01[80, c] = 1.0
        OH01[81, c] = e1

    CM = np.zeros((128, 1), np.float32)
    CM[16:64, 0] = 1.0

    HS = np.zeros((64, 2 * B_LOC), np.float32)
    TM = np.zeros((2 * B_LOC, HC), np.float32)
    for b in range(B_LOC):
        tstar = int(il[b]) - 1
        for i, sstar in enumerate([2 * int(tl[b]) - 1, 2 * int(tl[b])]):
            jstar = sstar // CORE
            p = sstar - CORE * jstar + HALO
            HS[p, b * 2 + i] = e1 if i == 0 else 1.0
            TM[b * 2 + i, (tstar - SNAP_T0) * (B_LOC * 3) + b * 3 + (jstar - 6)] = 1.0

    PS = np.zeros((2 * B_LOC, B_LOC), np.float32)
    for b in range(B_LOC):
        PS[2 * b, b] = 1.0
        PS[2 * b + 1, b] = 1.0

    C1IL = (C1 * il + CTILT * 2 * tl).astype(np.float32).reshape(B_LOC, 1)
    RTL = (1.0 / tl.astype(np.float64)).astype(np.float32).reshape(B_LOC, 1)
    MR = np.zeros((1, N_RESC * B_LOC), np.float32)
    for k in range(N_RESC):
        t = (k + 1) * RESC
        MR[0, k * B_LOC:(k + 1) * B_LOC] = (t < il).astype(np.float32)
    return dict(g=g_in, w1=W1, pr=PR, oh01=OH01, cmask=CM, hs=HS, tmask=TM,
                pairsel=PS, c1il=C1IL, rtl=RTL, maskr=MR,
                one1=np.ones((1, 1), np.float32))


def build_program(stage=5, nsteps=T):
    """Build the Bass/Tile program (shared by all cores).

    stage: 1=bulk only, 2=+dp, 3=+snapshots, 4=+rescale, 5=full (readout).
    Lower stages write a debug tile to y instead of the real readout.
    """
    import concourse.bacc as bacc
    import concourse.tile as tile
    from concourse import mybir

    f32 = mybir.dt.float32
    bf16 = mybir.dt.bfloat16
    AF = mybir.ActivationFunctionType
    ALU = mybir.AluOpType
    AX = mybir.AxisListType

    nc = bacc.Bacc("TRN2", target_bir_lowering=False, debug=False,
                   enable_asserts=False)

    pred = nc.dram_tensor("pred", [B_LOC, T, D], f32, kind="ExternalInput").ap()
    wt = nc.dram_tensor("wt", [D, V], f32, kind="ExternalInput").ap()
    bb = nc.dram_tensor("bb", [V, 1], f32, kind="ExternalInput").ap()
    g_d = nc.dram_tensor("g", [V, NCOL * 128], bf16, kind="ExternalInput").ap()
    w1_d = nc.dram_tensor("w1", [128, 128], f32, kind="ExternalInput").ap()
    pr_d = nc.dram_tensor("pr", [128, 80], f32, kind="ExternalInput").ap()
    oh01_d = nc.dram_tensor("oh01", [128, NCOL], f32, kind="ExternalInput").ap()
    cm_d = nc.dram_tensor("cmask", [128, 1], f32, kind="ExternalInput").ap()
    hs_d = nc.dram_tensor("hs", [64, 2 * B_LOC], f32, kind="ExternalInput").ap()
    tm_d = nc.dram_tensor("tmask", [2 * B_LOC, HC], f32, kind="ExternalInput").ap()
    ps_d = nc.dram_tensor("pairsel", [2 * B_LOC, B_LOC], f32, kind="ExternalInput").ap()
    c1il_d = nc.dram_tensor("c1il", [B_LOC, 1], f32, kind="ExternalInput").ap()
    rtl_d = nc.dram_tensor("rtl", [B_LOC, 1], f32, kind="ExternalInput").ap()
    mr_d = nc.dram_tensor("maskr", [1, N_RESC * B_LOC], f32, kind="ExternalInput").ap()
    one1_d = nc.dram_tensor("one1", [1, 1], f32, kind="ExternalInput").ap()
    y_d = nc.dram_tensor("y", [B_LOC, 1], f32, kind="ExternalOutput").ap()

    dma = nc.default_dma_engine.dma_start

    with tile.TileContext(nc) as tc:
        with (
            tc.tile_pool(name="consts", bufs=1) as cp,
            tc.tile_pool(name="persist", bufs=1) as pp,
            tc.tile_pool(name="xs", bufs=4) as xp,
            tc.tile_pool(name="accs", bufs=2) as accp,
            tc.tile_pool(name="small", bufs=2) as smp,
        ):
            wt_sb = cp.tile([128, 6, V], f32, tag="wt")
            dma(wt_sb, wt.rearrange("(c p) v -> p c v", p=128))
            bb_sb = cp.tile([V, 1], f32, tag="bb")
            dma(bb_sb, bb)
            g_sb = cp.tile([V, NCOL, 128], bf16, tag="g")
            dma(g_sb, g_d.rearrange("v (c m) -> v c m", m=128))
            w1_sb = cp.tile([128, 128], f32, tag="w1")
            dma(w1_sb, w1_d)
            pr_sb = cp.tile([128, 80], f32, tag="pr")
            dma(pr_sb, pr_d)
            oh01_sb = cp.tile([128, NCOL], f32, tag="oh01")
            dma(oh01_sb, oh01_d)
            cm_sb = cp.tile([128, 1], f32, tag="cmask")
            dma(cm_sb, cm_d)
            hs_sb = cp.tile([64, 2 * B_LOC], f32, tag="hs")
            dma(hs_sb, hs_d)
            tm_sb = cp.tile([2 * B_LOC, HC], f32, tag="tmask")
            dma(tm_sb, tm_d)
            ps_sb = cp.tile([2 * B_LOC, B_LOC], f32, tag="pairsel")
            dma(ps_sb, ps_d)
            c1il_sb = cp.tile([B_LOC, 1], f32, tag="c1il")
            dma(c1il_sb, c1il_d)
            rtl_sb = cp.tile([B_LOC, 1], f32, tag="rtl")
            dma(rtl_sb, rtl_d)
            mr_sb = cp.tile([1, N_RESC * B_LOC], f32, tag="maskr")
            dma(mr_sb, mr_d)
            one1_sb = cp.tile([1, 1], f32, tag="one1")
            dma(one1_sb, one1_d)
            ones_col = cp.tile([128, 1], f32, tag="ones_col")
            nc.vector.memset(ones_col, 1.0)
            ones_row = cp.tile([1, 128], f32, tag="ones_row")
            nc.vector.memset(ones_row, 1.0)
            c1b = cp.tile([1, 1], f32, tag="c1b")
            nc.vector.memset(c1b, float(C1))

            E_sb = pp.tile([V, B_LOC, 2, 500], f32, tag="E")
            P_sb = pp.tile([V, B_LOC, 2, 500], bf16, tag="P")
            PQ = pp.tile([128, NCOL, T], bf16, tag="PQ")
            HIST = pp.tile([64, NT, B_LOC, 3], f32, tag="HIST")

            # ---------------- bulk: logits -> E -> p -> gathered pq ----------
            with (
                tc.tile_pool(name="predp", bufs=3) as predp,
                tc.tile_pool(name="plog", bufs=2, space="PSUM") as plog,
                tc.tile_pool(name="pz", bufs=2, space="PSUM") as pzp,
                tc.tile_pool(name="przb", bufs=2, space="PSUM") as przbp,
                tc.tile_pool(name="pgat", bufs=2, space="PSUM") as pgatp,
            ):
                for b in range(B_LOC):
                    for h in range(2):
                        ps_log = plog.tile([V, 500], f32, tag="pslog")
                        for c in range(6):
                            pt = predp.tile([128, 500], f32, tag="pt")
                            dma(pt, pred[b, h * 500:(h + 1) * 500,
                                         c * 128:(c + 1) * 128].rearrange("t d -> d t"))
                            nc.tensor.matmul(ps_log, wt_sb[:, c, :], pt,
                                             start=(c == 0), stop=(c == 5))
                        Es = E_sb[:, b, h, :]
                        nc.scalar.activation(Es, ps_log, AF.Exp, bias=bb_sb)
                        ps_z = pzp.tile([1, 500], f32, tag="psz")
                        nc.tensor.matmul(ps_z, ones_col[0:V, :], Es,
                                         start=True, stop=True)
                        lz = smp.tile([1, 500], f32, tag="lz")
                        nc.scalar.activation(lz, ps_z, AF.Ln)
                        rz = smp.tile([1, 500], f32, tag="rz")
                        nc.scalar.activation(rz, lz, AF.Exp, bias=c1b,
                                             scale=-1.0)
                        ps_rzb = przbp.tile([V, 500], f32, tag="psrzb")
                        nc.tensor.matmul(ps_rzb, ones_row[:, 0:V], rz,
                                         start=True, stop=True)
                        nc.vector.tensor_mul(P_sb[:, b, h, :], Es, ps_rzb)
                        for j in range(NJ):
                            gidx = b * NJ + j
                            ps_g = pgatp.tile([128, 500], f32, tag="psg")
                            nc.tensor.matmul(ps_g, g_sb[:, gidx, :],
                                             P_sb[:, b, h, :],
                                             start=True, stop=True)
                            dst = PQ[:, gidx, h * 500:(h + 1) * 500]
                            if j % 2 == 0:
                                nc.scalar.copy(dst, ps_g)
                            else:
                                nc.vector.tensor_copy(dst, ps_g)

            # ---------------- DP over time ----------------------------------
            with (
                tc.tile_pool(name="pstep", bufs=2, space="PSUM") as pstepp,
                tc.tile_pool(name="phalo", bufs=2, space="PSUM") as phalop,
                tc.tile_pool(name="ps36", bufs=1, space="PSUM") as ps36p,
                tc.tile_pool(name="pbr", bufs=1, space="PSUM") as pbrp,
            ):
                X = xp.tile([128, NCOL], f32, tag="X")
                nc.vector.tensor_mul(X, PQ[:, :, 0], oh01_sb)
                acc = accp.tile([1, B_LOC], f32, tag="acc")
                nc.vector.memset(acc, 0.0)
                for t in range(1, nsteps):
                    if t % RESC == 0 and stage >= 4:
                        k = t // RESC - 1
                        ps36 = ps36p.tile([1, NCOL], f32, tag="ps36")
                        nc.tensor.matmul(ps36, cm_sb, X, start=True, stop=True)
                        s4 = smp.tile([1, B_LOC], f32, tag="s4")
                        nc.vector.tensor_reduce(
                            s4, ps36.rearrange("p (b j) -> p b j", j=NJ),
                            axis=AX.X, op=ALU.add)
                        ls4 = smp.tile([1, B_LOC], f32, tag="ls4")
                        nc.scalar.activation(ls4, s4, AF.Ln)
                        spk = smp.tile([1, B_LOC], f32, tag="spk")
                        nc.vector.tensor_mul(
                            spk, ls4, mr_sb[:, k * B_LOC:(k + 1) * B_LOC])
                        acc2 = accp.tile([1, B_LOC], f32, tag="acc")
                        nc.vector.tensor_add(acc2, acc, spk)
                        acc = acc2
                        rz4 = smp.tile([1, B_LOC], f32, tag="rz4")
                        nc.scalar.activation(rz4, ls4, AF.Exp, scale=-1.0)
                        ps_br = pbrp.tile([128, B_LOC], f32, tag="psbr")
                        nc.tensor.matmul(ps_br, ones_row, rz4,
                                         start=True, stop=True)
                        br4 = smp.tile([128, B_LOC], f32, tag="br4")
                        nc.vector.tensor_copy(br4, ps_br)
                        Xr = xp.tile([128, NCOL], f32, tag="X")
                        for b in range(B_LOC):
                            nc.vector.tensor_scalar_mul(
                                Xr[:, b * NJ:(b + 1) * NJ],
                                X[:, b * NJ:(b + 1) * NJ], br4[:, b:b + 1])
                        X = Xr
                    ps = pstepp.tile([128, NCOL], f32, tag="ps")
                    nc.tensor.matmul(ps, w1_sb, X, start=True, stop=True)
                    Xt = xp.tile([128, NCOL], f32, tag="X")
                    nc.vector.tensor_mul(Xt, ps, PQ[:, :, t])
                    if t % REFRESH == 0:
                        psh = phalop.tile([80, 32], f32, tag="psh")
                        xv = Xt.rearrange("p (b j) -> p b j", j=NJ)
                        nc.tensor.matmul(psh, pr_sb, xv[:, :, 0:8],
                                         start=True, stop=True)
                        phv = psh.rearrange("p (b j) -> p b j", j=8)
                        nc.vector.tensor_copy(xv[0:16, :, 1:9], phv[0:16])
                        nc.vector.tensor_copy(xv[64:80, :, 1:9], phv[64:80])
                    if t >= SNAP_T0 and stage >= 3:
                        nc.scalar.copy(
                            HIST[:, t - SNAP_T0, :, :],
                            Xt.rearrange("p (b j) -> p b j", j=NJ)[0:64, :, 6:9])
                    X = Xt

            if stage <= 4:
                dbg = smp.tile([B_LOC, 1], f32, tag="dbg")
                nc.vector.tensor_copy(dbg, X[0:B_LOC, 0:1])
                dma(y_d, dbg)

            # ---------------- readout ---------------------------------------
            with (
                tc.tile_pool(name="pread", bufs=2, space="PSUM") as preadp,
                tc.tile_pool(name="ppair", bufs=1, space="PSUM") as ppairp,
                tc.tile_pool(name="pacc4", bufs=1, space="PSUM") as pacc4p,
            ):
                if stage < 5:
                    hist_flat = None  # readout disabled at this stage
                    HC_eff = 0
                else:
                    hist_flat = HIST.rearrange("p t b j -> p (t b j)")
                    HC_eff = HC
                vals = None
                k0 = 0
                while k0 < HC_eff:
                    n = min(500, HC - k0)
                    ps_r = preadp.tile([2 * B_LOC, 500], f32, tag="psr")
                    nc.tensor.matmul(ps_r[:, :n], hs_sb,
                                     hist_flat[:, k0:k0 + n],
                                     start=True, stop=True)
                    mt = smp.tile([2 * B_LOC, 500], f32, tag="mt")
                    nc.vector.tensor_mul(mt[:, :n], ps_r[:, :n],
                                         tm_sb[:, k0:k0 + n])
                    red = smp.tile([2 * B_LOC, 1], f32, tag="red")
                    nc.vector.tensor_reduce(red, mt[:, :n], axis=AX.X,
                                            op=ALU.add)
                    if vals is None:
                        vals = red
                    else:
                        v2 = smp.tile([2 * B_LOC, 1], f32, tag="vals")
                        nc.vector.tensor_add(v2, vals, red)
                        vals = v2
                    k0 += n
                if stage >= 5:
                    ps_pair = ppairp.tile([B_LOC, 1], f32, tag="pspair")
                    nc.tensor.matmul(ps_pair, ps_sb, vals, start=True, stop=True)
                    la = smp.tile([B_LOC, 1], f32, tag="la")
                    nc.scalar.activation(la, ps_pair, AF.Ln)
                    ps_a4 = pacc4p.tile([B_LOC, 1], f32, tag="psa4")
                    nc.tensor.matmul(ps_a4, acc, one1_sb, start=True, stop=True)
                    t1 = smp.tile([B_LOC, 1], f32, tag="t1")
                    nc.vector.tensor_sub(t1, c1il_sb, ps_a4)
                    t2 = smp.tile([B_LOC, 1], f32, tag="t2")
                    nc.vector.tensor_sub(t2, t1, la)
                    t3 = smp.tile([B_LOC, 1], f32, tag="t3")
                    nc.vector.tensor_mul(t3, t2, rtl_sb)
                    dma(y_d, t3)

    nc.compile()
    return nc


def build_in_maps(inputs):
    """Shard inputs + host-built constants -> one in_map per core."""
    pred = np.ascontiguousarray(np.asarray(inputs["pred"], np.float32))
    targets = np.asarray(inputs["targets"]).astype(np.int64)
    in_len = np.asarray(inputs["input_lengths"]).astype(np.int64)
    tgt_len = np.asarray(inputs["target_lengths"]).astype(np.int64)
    Wm = np.asarray(inputs["W"], np.float32)
    bv = np.asarray(inputs["b"], np.float32)
    tgt2d = targets.reshape(B, L)
    wt = np.ascontiguousarray(Wm.T)                    # (768, 31)
    bb = np.ascontiguousarray(bv.reshape(V, 1))
    in_maps = []
    for core in range(8):
        b0 = core * B_LOC
        cst = _build_core_consts(tgt2d, in_len, tgt_len, b0)
        im = dict(pred=np.ascontiguousarray(pred[b0:b0 + B_LOC]),
                  wt=wt, bb=bb)
        for k, v in cst.items():
            im[k] = np.ascontiguousarray(v)
        in_maps.append(im)
    return in_maps


_CACHED = {}


def kernel(**inputs):
    from concourse import bass_utils
    if "nc" not in _CACHED:
        _CACHED["nc"] = build_program()
    nc = _CACHED["nc"]
    in_maps = build_in_maps(inputs)
    res = bass_utils.run_bass_kernel_spmd(nc, in_maps, core_ids=list(range(8)))
    ys = [r["y"] for r in res.results]
    loss = np.concatenate([y.ravel() for y in ys]).astype(np.float64).sum() / B
    return np.float32(loss)

